# revision 1
# baseline (speedup 1.0000x reference)
"""Bernstein flow density kernel for Trainium2 (8 NeuronCores, data-parallel).

Math (per sample x in R^5, per dim i):
  c = constrained(A_i)                     # [(4)^i, 15] monotone coeffs in (0,1)
  tf_k = sum_j cb_ij c[j,k]                # cb_i = multivariate Bernstein basis over x[:i]
  dcoef_k = tf_k - tf_{k-1}  (tf_{-1}=0, tf_15=1)
  db_k = 16*comb(15,k) x_i^k (1-x_i)^(15-k)
  f_i = sum_k dcoef_k db_k ;  density = prod_i f_i

Device mapping:
  - dcoef directly from matmul: fold the k-difference into the weight matrix
    (column diffs of c; last column = comb - c[:,14] using partition of unity).
  - db via exp(k*ln x + (15-k)*ln u + ln(16 comb)) : one small matmul over
    [ln x; ln u] (fp16 hi+lo split for accuracy) + one ACT Exp.
  - cb built on DVE in fp16 (pure monomials, comb(3,.) folded into weights),
    transposed to basis-major layout with the DMA xbar (2-byte transpose).
  - f_i  = ones-block matmul over dcoef*db rows; density = exp(ones-matmul of ln f).
"""

import math
import sys

import numpy as np

for _p in ("/opt/trn_rl_repo", "/root/.axon_site/_ro/trn_rl_repo"):
    if _p not in sys.path:
        sys.path.append(_p)

import concourse.bass as bass
import concourse.tile as tile
from concourse import bacc, mybir
from concourse.bass_utils import run_bass_kernel_spmd

F32 = mybir.dt.float32
F16 = mybir.dt.float16
F32R = mybir.dt.float32r

DIM = 5
TF_DEG = 16
N_FULL = 262144
N_CORES = 8
N_CORE = N_FULL // N_CORES  # 32768
SC = 256.0  # scale folded into dcoef weights to keep fp16 away from subnormals
COMB3 = np.array([1.0, 3.0, 3.0, 1.0])
COMB15 = np.array([math.comb(15, k) for k in range(16)], dtype=np.float64)


# ----------------------------------------------------------------- host consts
def _constrained(A):
    A = A.astype(np.float64)
    sp = np.log1p(np.exp(-np.abs(A))) + np.maximum(A, 0.0)  # softplus, stable
    cs = np.cumsum(sp, axis=1)
    return 2.0 * (1.0 / (1.0 + np.exp(-cs)) - 0.5)


def _dev_perm_scale(i):
    """Map device row p (j_{i-1} slowest ... j_0 fastest reversed: p = sum_d j_d*4^d)
    to reference row (j_0 slowest: ref = sum_d j_d*4^(i-1-d)) + comb scale."""
    rows = 4**i
    ref_idx = np.zeros(rows, dtype=np.int64)
    scale = np.ones(rows)
    for p in range(rows):
        r = 0
        s = 1.0
        for d in range(i):  # j_d = digit d of p (j_0 = fastest)
            jd = (p >> (2 * d)) & 3
            r += jd * 4 ** (i - 1 - d)
            s *= COMB3[jd]
        ref_idx[p] = r
        scale[p] = s
    return ref_idx, scale


def _dcoef_weights(C, combscale):
    """C: [rows,15] device-row-ORDERED original coeffs (not comb-divided);
    returns [rows,16] W with the tf-difference folded in, scaled so that
    sum_j monomial_j * W[j,k] = SC * dcoef_k."""
    rows = C.shape[0]
    W = np.zeros((rows, 16))
    W[:, 0] = C[:, 0]
    W[:, 1:15] = C[:, 1:15] - C[:, 0:14]
    W[:, 15] = 1.0 - C[:, 14]
    return W * combscale[:, None] * SC


def build_consts(A_list):
    Cs = []
    for i in range(DIM):
        C = _constrained(A_list[i])  # [(4)^i, 15] in reference row order
        if i == 0:
            Cs.append((C, np.ones(1)))
        else:
            ref_idx, scale = _dev_perm_scale(i)
            # device row p uses reference row ref_idx[p]; comb folded via scale
            Cs.append((C[ref_idx], scale))
    Wd = [_dcoef_weights(Cperm, scale) for (Cperm, scale) in Cs]  # [rows,16] each

    # fp16 dynamic range fix: scale each (dim>=1, k) weight column by a power
    # of two so its max lands near 1024 (away from fp16 subnormals), and fold
    # the inverse into the exp bias of the matching db row (exact compensation).
    colshift = np.zeros((5, 16))  # ln of the applied scale, for expbias
    for i in range(5):
        m = np.max(np.abs(Wd[i]), axis=0)  # [16]
        e = np.round(np.log2(1024.0 / np.maximum(m, 1e-300)))
        e = np.clip(e, -10, 40)
        s = np.exp2(e)
        Wd[i] = Wd[i] * s[None, :]
        colshift[i] = e * math.log(2.0)

    w13 = np.zeros((84, 64))
    w13[0:4, 0:16] = Wd[1]
    w13[4:20, 16:32] = Wd[2]
    w13[20:84, 32:48] = Wd[3]
    w4a = Wd[4][0:128]  # [128,16]
    w4b = Wd[4][128:256]

    # Wlog: rhs rows q (0:20) -> [lnx_d hi (0:5), lnu_d hi (5:10), lnx lo, lnu lo]
    # out rows r (0:96): 0:48 d=1..3 k=0:16 ; 48:64 zero ; 64:80 d=4 ; 80:96 d=0
    wlog = np.zeros((20, 96))

    def _rowmap(r):
        if r < 48:
            return 1 + r // 16, r % 16
        if r < 64:
            return None
        if r < 80:
            return 4, r - 64
        return 0, r - 80

    for r in range(96):
        mk = _rowmap(r)
        if mk is None:
            continue
        d, k = mk
        for base in (0, 10):  # hi rows and lo rows share coefficients
            wlog[base + d, r] = float(k)
            wlog[base + 5 + d, r] = float(15 - k)
    # single band-masked copy: ln values ride inside the cbA region at rows
    # 84:104 after the block transpose; K=128 matmul with zeros elsewhere
    wlog1 = np.zeros((128, 96))
    wlog1[84:104, :] = wlog

    expbias = np.zeros((96, 1))
    for r in range(96):
        mk = _rowmap(r)
        if mk is None:
            continue
        d, k = mk
        expbias[r, 0] = math.log(16.0 * COMB15[k]) - colshift[d, k]

    f1w = np.zeros((80, 32))
    for i in (1, 2, 3):
        f1w[(i - 1) * 16 : i * 16, i] = 1.0
    f1w[64:80, 4] = 1.0

    f2w = np.zeros((96, 32))
    f2w[80:96, 0] = Wd[0][0]  # dcoef weights of dim 0 applied to db d=0 rows
    f2w[48, 5:32] = 1.0  # pad cols read dbT row 48 (==1.0): psum pad stays ln-safe

    lnones = np.zeros((128, 4))
    for t in range(4):
        lnones[32 * t : 32 * t + 5, t] = 1.0

    fbias = np.full((4, 1), -DIM * math.log(SC))

    return {
        "fbias": fbias.astype(np.float32),
        "w13": w13.astype(np.float16),
        "w4a": w4a.astype(np.float16),
        "w4b": w4b.astype(np.float16),
        "wlog1": wlog1.astype(np.float16),
        "expbias": expbias.astype(np.float32),
        "f1w": f1w.astype(np.float16),
        "f2w": f2w.astype(np.float16),
        "lnones": lnones.astype(np.float32),
    }


# ---------------------------------------------------------------- device build
def _ap(t, extra_offset, dims):
    """Manual AP over a tile: keep its partition dim, custom free dims."""
    return bass.AP(
        tensor=t.tensor, offset=t.offset + extra_offset, ap=[list(t.ap[0])] + dims
    )


def build_nc(ncore, nblk):
    """nblk = sub-tiles (128 samples each) per block; must be mult of 16."""
    assert nblk % 16 == 0
    nsamp_blk = 128 * nblk
    assert ncore % nsamp_blk == 0
    nblocks = ncore // nsamp_blk
    ngroups = nblk // 4  # 512-sample groups per block
    xcols = ncore // 128 * DIM

    nc = bacc.Bacc("TRN2", target_bir_lowering=False, debug=False, num_devices=N_CORES)
    xt = nc.declare_dram_parameter("xt", [128, xcols], F32, isOutput=False)
    w13 = nc.declare_dram_parameter("w13", [84, 64], F16, isOutput=False)
    w4a = nc.declare_dram_parameter("w4a", [128, 16], F16, isOutput=False)
    w4b = nc.declare_dram_parameter("w4b", [128, 16], F16, isOutput=False)
    wlog1 = nc.declare_dram_parameter("wlog1", [128, 96], F16, isOutput=False)
    expbias = nc.declare_dram_parameter("expbias", [96, 1], F32, isOutput=False)
    f1w = nc.declare_dram_parameter("f1w", [80, 32], F16, isOutput=False)
    f2w = nc.declare_dram_parameter("f2w", [96, 32], F16, isOutput=False)
    lnones = nc.declare_dram_parameter("lnones", [128, 4], F32, isOutput=False)
    fbias = nc.declare_dram_parameter("fbias", [4, 1], F32, isOutput=False)
    dens = nc.declare_dram_parameter("dens", [ncore], F32, isOutput=True)

    Exp = mybir.ActivationFunctionType.Exp
    Ln = mybir.ActivationFunctionType.Ln

    with tile.TileContext(nc) as tc:
        with (
            tc.tile_pool(name="wc", bufs=1) as wc,
            tc.tile_pool(name="la", bufs=2) as la,
            tc.tile_pool(name="gr", bufs=3) as gr,
            tc.tile_pool(name="tr", bufs=2) as tr,
            tc.tile_pool(name="sb", bufs=2) as sbp,
            tc.tile_pool(name="psg", bufs=2, space="PSUM") as psg,
            tc.tile_pool(name="psf", bufs=2, space="PSUM") as psf,
            tc.tile_pool(name="psd", bufs=2, space="PSUM") as psd,
        ):
            w13sb = wc.tile([84, 64], F16, tag="w13")
            w4asb = wc.tile([128, 16], F16, tag="w4a")
            w4bsb = wc.tile([128, 16], F16, tag="w4b")
            wlogsb = wc.tile([128, 96], F16, tag="wlog")
            expbsb = wc.tile([96, 1], F32, tag="expb")
            f1wsb = wc.tile([80, 32], F16, tag="f1w")
            f2wsb = wc.tile([96, 32], F16, tag="f2w")
            lnosb = wc.tile([128, 4], F32, tag="lno")
            fbsb = wc.tile([4, 1], F32, tag="fb")
            xall = wc.tile([128, xcols], F32, tag="xall")
            for dst, src in (
                (w13sb, w13),
                (w4asb, w4a),
                (w4bsb, w4b),
                (wlogsb, wlog1),
                (expbsb, expbias),
                (f1wsb, f1w),
                (f2wsb, f2w),
                (lnosb, lnones),
                (fbsb, fbias),
                (xall, xt),
            ):
                nc.gpsimd.dma_start(out=dst[:], in_=src[:])

            for blk in range(nblocks):
                n = nblk
                xa = xall[:, blk * n * 5 : (blk + 1) * n * 5].rearrange(
                    "p (n d) -> p n d", d=5
                )
                u = la.tile([128, n, 4], F32, tag="u")
                xp2 = la.tile([128, n, 4], F32, tag="xp2")
                up2 = la.tile([128, n, 4], F32, tag="up2")
                ln32 = la.tile([128, n, 10], F32, tag="ln32")
                b4 = la.tile([128, n, 4, 4], F16, tag="b4")
                # per-sub-tile 128-col regions (contiguous -> one block transpose each)
                # cbA: 0:4 cb1 | 4:20 cb2 | 20:84 cb3 | 84:104 ln hi/lo | 104:128 zero
                cbA = la.tile([128, n, 128], F16, tag="cbA")
                cbB = la.tile([128, n, 128], F16, tag="cbB")  # cb4 rows 0:128
                cbC = la.tile([128, n, 128], F16, tag="cbC")  # cb4 rows 128:256

                x4 = xa[:, :, 0:4]
                nc.vector.tensor_scalar(
                    out=u[:],
                    in0=x4,
                    scalar1=1.0,
                    scalar2=-1.0,
                    op0=mybir.AluOpType.subtract,
                    op1=mybir.AluOpType.mult,
                )
                nc.vector.tensor_mul(out=xp2[:], in0=x4, in1=x4)
                nc.vector.tensor_mul(out=up2[:], in0=u[:], in1=u[:])
                nc.scalar.activation(out=ln32[:, :, 0:5], in_=xa, func=Ln)
                nc.scalar.activation(
                    out=ln32[:, :, 5:10], in_=xa, func=Ln, scale=-1.0, bias=1.0
                )
                nc.vector.tensor_copy(out=cbA[:, :, 84:94], in_=ln32[:])
                nc.vector.tensor_sub(
                    out=cbA[:, :, 94:104], in0=ln32[:], in1=cbA[:, :, 84:94]
                )
                nc.vector.memset(cbA[:, :, 104:128], 0.0)
                # b4[:, :, j, d]: j0=u^3, j1=x u^2, j2=x^2 u, j3=x^3 (d=0..3)
                nc.vector.tensor_mul(out=b4[:, :, 0, :], in0=up2[:], in1=u[:])
                nc.vector.tensor_mul(out=b4[:, :, 1, :], in0=x4, in1=up2[:])
                nc.vector.tensor_mul(out=b4[:, :, 2, :], in0=xp2[:], in1=u[:])
                nc.vector.tensor_mul(out=b4[:, :, 3, :], in0=xp2[:], in1=x4)
                nc.vector.tensor_copy(
                    out=cbA[:, :, 0:4], in_=_ap(b4[:], 0, [[16, n], [4, 4]])
                )
                nc.vector.tensor_mul(
                    out=cbA[:, :, 4:20].rearrange("p n (a b) -> p n a b", a=4),
                    in0=_ap(b4[:], 0, [[16, n], [0, 4], [4, 4]]),
                    in1=_ap(b4[:], 1, [[16, n], [4, 4], [0, 4]]),
                )
                nc.vector.tensor_mul(
                    out=cbA[:, :, 20:84].rearrange("p n (a b) -> p n a b", a=4),
                    in0=_ap(cbA[:], 4, [[128, n], [0, 4], [1, 16]]),
                    in1=_ap(b4[:], 2, [[16, n], [4, 4], [0, 16]]),
                )
                nc.vector.tensor_mul(
                    out=cbB[:].rearrange("p n (a b) -> p n a b", a=2),
                    in0=_ap(cbA[:], 20, [[128, n], [0, 2], [1, 64]]),
                    in1=_ap(b4[:], 3, [[16, n], [4, 2], [0, 64]]),
                )
                nc.gpsimd.tensor_mul(
                    out=cbC[:].rearrange("p n (a b) -> p n a b", a=2),
                    in0=_ap(cbA[:], 20, [[128, n], [0, 2], [1, 64]]),
                    in1=_ap(b4[:], 11, [[16, n], [4, 2], [0, 64]]),
                )

                # one batched xbar transpose per region per block:
                # out[:, j, :] = transpose(in[:, j*128:(j+1)*128])
                cbTA = tr.tile([128, n, 128], F16, tag="cbTA")
                cbTB = tr.tile([128, n, 128], F16, tag="cbTB")
                cbTC = tr.tile([128, n, 128], F16, tag="cbTC")
                for dst_t, src_t in ((cbTA, cbA), (cbTB, cbB), (cbTC, cbC)):
                    nc.sync.dma_start(
                        out=dst_t[:],
                        in_=src_t[:].rearrange("p n c -> p (n c)"),
                        transpose=True,
                    )

                for sb in range(ngroups // 4):  # superblock: 4 groups / 2048 samples
                    fpsum = psf.tile([128, 512], F32, tag="fpsum")
                    for tp in range(4):
                        g = sb * 4 + tp
                        gsl = slice(4 * g, 4 * g + 4)
                        wlogp = psg.tile([96, 512], F32, tag="wlogp")
                        for t in range(4):
                            nc.tensor.matmul(
                                out=wlogp[:, t * 128 : (t + 1) * 128],
                                lhsT=wlogsb[:],
                                rhs=cbTA[:, 4 * g + t, :],
                                start=True,
                                stop=True,
                            )
                        dbT = gr.tile([96, 512], F16, tag="dbT")
                        nc.scalar.activation(
                            out=dbT[:], in_=wlogp[:], func=Exp, bias=expbsb[:]
                        )
                        dtfp = psg.tile([96, 512], F32, tag="dtfp")
                        nc.tensor.matmul(
                            out=dtfp[0:64, :],
                            lhsT=w13sb[:],
                            rhs=cbTA[0:84, gsl, :],
                            start=True,
                            stop=True,
                        )
                        nc.tensor.matmul(
                            out=dtfp[64:80, :],
                            lhsT=w4asb[:],
                            rhs=cbTB[:, gsl, :],
                            start=True,
                            stop=False,
                        )
                        nc.tensor.matmul(
                            out=dtfp[64:80, :],
                            lhsT=w4bsb[:],
                            rhs=cbTC[:, gsl, :],
                            start=False,
                            stop=True,
                        )
                        prod = gr.tile([80, 512], F16, tag="prod")
                        nc.vector.tensor_mul(
                            out=prod[:], in0=dtfp[0:80, :], in1=dbT[0:80, :]
                        )
                        frows = slice(32 * tp, 32 * tp + 32)
                        nc.tensor.matmul(
                            out=fpsum[frows, :],
                            lhsT=f1wsb[:],
                            rhs=prod[:],
                            start=True,
                            stop=False,
                            tile_position=(0, 32 * tp),
                        )
                        nc.tensor.matmul(
                            out=fpsum[frows, :],
                            lhsT=f2wsb[:],
                            rhs=dbT[:],
                            start=False,
                            stop=True,
                            tile_position=(0, 32 * tp),
                        )
                    lnf = sbp.tile([128, 512], F32, tag="lnf")
                    nc.scalar.activation(out=lnf[:], in_=fpsum[:], func=Ln)
                    lnden = psd.tile([4, 512], F32, tag="lnden")
                    nc.tensor.matmul(
                        out=lnden[:],
                        lhsT=lnosb[:],
                        rhs=lnf[:],
                        start=True,
                        stop=True,
                    )
                    dens_sb = sbp.tile([4, 512], F32, tag="dens_sb")
                    nc.scalar.activation(
                        out=dens_sb[:],
                        in_=lnden[:],
                        func=Exp,
                        bias=fbsb[:],
                    )
                    base = blk * nsamp_blk + sb * 2048
                    nc.gpsimd.dma_start(
                        out=dens[base : base + 2048].rearrange("(t s) -> t s", t=4),
                        in_=dens_sb[:],
                    )
    nc.finalize()
    return nc


# -------------------------------------------------------------------- host run
def pack_x(x_shard):
    """[N_CORE, 5] -> [128, N_CORE/128*5]; sample s = nb*128+p -> row p, cols nb*5+d."""
    n = x_shard.shape[0]
    return (
        np.ascontiguousarray(x_shard.reshape(n // 128, 128, 5).transpose(1, 0, 2))
        .reshape(128, n // 128 * 5)
        .astype(np.float32)
    )


_CACHE = {}


def _get_runner():
    """Build nc + a cached jitted shard_map callable (trace/compile once)."""
    if "runner" in _CACHE:
        return _CACHE["runner"]
    import jax
    from jax.sharding import Mesh, PartitionSpec
    from jax.experimental.shard_map import shard_map

    from concourse import bass2jax, mybir as _mb
    from concourse.bass2jax import (
        _bass_exec_p,
        install_neuronx_cc_hook,
        partition_id_tensor,
    )

    install_neuronx_cc_hook()
    nc = build_nc(N_CORE, 32)
    partition_name = nc.partition_id_tensor.name if nc.partition_id_tensor else None

    in_names, out_names, out_avals, zero_outs = [], [], [], []
    for alloc in nc.m.functions[0].allocations:
        if not isinstance(alloc, _mb.MemoryLocationSet):
            continue
        name = alloc.memorylocations[0].name
        if alloc.kind == "ExternalInput":
            if name != partition_name:
                in_names.append(name)
        elif alloc.kind == "ExternalOutput":
            out_names.append(name)
            shape = tuple(alloc.tensor_shape)
            dtype = _mb.dt.np(alloc.dtype)
            out_avals.append(jax.core.ShapedArray(shape, dtype))
            zero_outs.append(np.zeros(shape, dtype))
    n_params = len(in_names)
    all_in_names = list(in_names) + list(out_names)
    if partition_name is not None:
        all_in_names.append(partition_name)

    def _body(*args):
        operands = list(args)
        if partition_name is not None:
            operands.append(partition_id_tensor())
        outs = _bass_exec_p.bind(
            *operands,
            out_avals=tuple(out_avals),
            in_names=tuple(all_in_names),
            out_names=tuple(out_names),
            lowering_input_output_aliases=(),
            sim_require_finite=True,
            sim_require_nnan=True,
            nc=nc,
        )
        return tuple(outs)

    devices = jax.devices()[:N_CORES]
    mesh = Mesh(np.asarray(devices), ("core",))
    in_specs = (PartitionSpec("core"),) * (n_params + len(out_names))
    out_specs = (PartitionSpec("core"),) * len(out_names)
    sharded = jax.jit(
        shard_map(
            _body, mesh=mesh, in_specs=in_specs, out_specs=out_specs, check_rep=False
        ),
        keep_unused=True,
    )
    shard = jax.NamedSharding(mesh, PartitionSpec("core"))
    zeros_dev = [
        jax.device_put(
            np.zeros((N_CORES * z.shape[0], *z.shape[1:]), z.dtype), shard
        )
        for z in zero_outs
    ]
    _CACHE["runner"] = (sharded, in_names, out_names, out_avals, zeros_dev, shard)
    return _CACHE["runner"]


def run_device(in_maps):
    """in_maps: per-core dicts. Returns list of per-core output dicts."""
    import jax

    sharded, in_names, out_names, out_avals, zeros_dev, shard = _get_runner()
    concat_in = [
        jax.device_put(
            np.concatenate(
                [np.asarray(in_maps[c][k]) for c in range(N_CORES)], axis=0
            ),
            shard,
        )
        for k in in_names
    ]
    out_arrs = sharded(*concat_in, *zeros_dev)
    return [
        {
            k: np.asarray(out_arrs[i]).reshape(N_CORES, *out_avals[i].shape)[c]
            for i, k in enumerate(out_names)
        }
        for c in range(N_CORES)
    ]


def make_in_maps(x, A_list):
    consts = build_consts([np.asarray(a) for a in A_list])
    in_maps = []
    for c in range(N_CORES):
        m = {"xt": pack_x(x[c * N_CORE : (c + 1) * N_CORE])}
        m.update(consts)
        in_maps.append(m)
    return in_maps


def kernel(x, A0, A1, A2, A3, A4):
    x = np.asarray(x, dtype=np.float32)
    in_maps = make_in_maps(x, (A0, A1, A2, A3, A4))
    res = run_device(in_maps)
    return np.concatenate([res[c]["dens"] for c in range(N_CORES)])



# revision 5
# speedup vs baseline: 27.8885x; 27.8885x over previous
"""Bernstein flow density kernel for Trainium2 (8 NeuronCores, data-parallel).

Math (per sample x in R^5, per dim i):
  c = constrained(A_i)                     # [(4)^i, 15] monotone coeffs in (0,1)
  tf_k = sum_j cb_ij c[j,k]                # cb_i = multivariate Bernstein basis over x[:i]
  dcoef_k = tf_k - tf_{k-1}  (tf_{-1}=0, tf_15=1)
  db_k = 16*comb(15,k) x_i^k (1-x_i)^(15-k)
  f_i = sum_k dcoef_k db_k ;  density = prod_i f_i

Device mapping:
  - dcoef directly from matmul: fold the k-difference into the weight matrix
    (column diffs of c; last column = comb - c[:,14] using partition of unity).
  - db via exp(k*ln x + (15-k)*ln u + ln(16 comb)) : one small matmul over
    [ln x; ln u] (fp16 hi+lo split for accuracy) + one ACT Exp.
  - cb built on DVE in fp16 (pure monomials, comb(3,.) folded into weights),
    transposed to basis-major layout with the DMA xbar (2-byte transpose).
  - f_i  = ones-block matmul over dcoef*db rows; density = exp(ones-matmul of ln f).
"""

import math
import sys

import numpy as np

for _p in ("/opt/trn_rl_repo", "/root/.axon_site/_ro/trn_rl_repo"):
    if _p not in sys.path:
        sys.path.append(_p)

import concourse.bass as bass
import concourse.tile as tile
from concourse import bacc, mybir
from concourse.bass_utils import run_bass_kernel_spmd

F32 = mybir.dt.float32
F16 = mybir.dt.float16
F32R = mybir.dt.float32r

DIM = 5
TF_DEG = 16
N_FULL = 262144
N_CORES = 8
N_CORE = N_FULL // N_CORES  # 32768
SC = 256.0  # scale folded into dcoef weights to keep fp16 away from subnormals
COMB3 = np.array([1.0, 3.0, 3.0, 1.0])
COMB15 = np.array([math.comb(15, k) for k in range(16)], dtype=np.float64)


# ----------------------------------------------------------------- host consts
def _constrained(A):
    A = A.astype(np.float64)
    sp = np.log1p(np.exp(-np.abs(A))) + np.maximum(A, 0.0)  # softplus, stable
    cs = np.cumsum(sp, axis=1)
    return 2.0 * (1.0 / (1.0 + np.exp(-cs)) - 0.5)


def _dev_perm_scale(i):
    """Map device row p (j_{i-1} slowest ... j_0 fastest reversed: p = sum_d j_d*4^d)
    to reference row (j_0 slowest: ref = sum_d j_d*4^(i-1-d)) + comb scale."""
    rows = 4**i
    ref_idx = np.zeros(rows, dtype=np.int64)
    scale = np.ones(rows)
    for p in range(rows):
        r = 0
        s = 1.0
        for d in range(i):  # j_d = digit d of p (j_0 = fastest)
            jd = (p >> (2 * d)) & 3
            r += jd * 4 ** (i - 1 - d)
            s *= COMB3[jd]
        ref_idx[p] = r
        scale[p] = s
    return ref_idx, scale


def _dcoef_weights(C, combscale):
    """C: [rows,15] device-row-ORDERED original coeffs (not comb-divided);
    returns [rows,16] W with the tf-difference folded in, scaled so that
    sum_j monomial_j * W[j,k] = SC * dcoef_k."""
    rows = C.shape[0]
    W = np.zeros((rows, 16))
    W[:, 0] = C[:, 0]
    W[:, 1:15] = C[:, 1:15] - C[:, 0:14]
    W[:, 15] = 1.0 - C[:, 14]
    return W * combscale[:, None] * SC


def build_consts(A_list):
    Cs = []
    for i in range(DIM):
        C = _constrained(A_list[i])  # [(4)^i, 15] in reference row order
        if i == 0:
            Cs.append((C, np.ones(1)))
        else:
            ref_idx, scale = _dev_perm_scale(i)
            # device row p uses reference row ref_idx[p]; comb folded via scale
            Cs.append((C[ref_idx], scale))
    Wd = [_dcoef_weights(Cperm, scale) for (Cperm, scale) in Cs]  # [rows,16] each

    # fp16 dynamic range fix: scale each (dim>=1, k) weight column by a power
    # of two so its max lands near 1024 (away from fp16 subnormals), and fold
    # the inverse into the exp bias of the matching db row (exact compensation).
    colshift = np.zeros((5, 16))  # ln of the applied scale, for expbias
    for i in range(5):
        m = np.max(np.abs(Wd[i]), axis=0)  # [16]
        e = np.round(np.log2(1024.0 / np.maximum(m, 1e-300)))
        e = np.clip(e, -10, 40)
        s = np.exp2(e)
        Wd[i] = Wd[i] * s[None, :]
        colshift[i] = e * math.log(2.0)

    w13 = np.zeros((84, 64))
    w13[0:4, 0:16] = Wd[1]
    w13[4:20, 16:32] = Wd[2]
    w13[20:84, 32:48] = Wd[3]
    w4a = Wd[4][0:128]  # [128,16]
    w4b = Wd[4][128:256]

    # Wlog: rhs rows q (0:20) -> [lnx_d hi (0:5), lnu_d hi (5:10), lnx lo, lnu lo]
    # out rows r (0:96): 0:48 d=1..3 k=0:16 ; 48:64 zero ; 64:80 d=4 ; 80:96 d=0
    wlog = np.zeros((20, 96))

    def _rowmap(r):
        if r < 48:
            return 1 + r // 16, r % 16
        if r < 64:
            return None
        if r < 80:
            return 4, r - 64
        return 0, r - 80

    for r in range(96):
        mk = _rowmap(r)
        if mk is None:
            continue
        d, k = mk
        for base in (0, 10):  # hi rows and lo rows share coefficients
            wlog[base + d, r] = float(k)
            wlog[base + 5 + d, r] = float(15 - k)
    # single band-masked copy: ln values ride inside the cbA region at rows
    # 84:104 after the block transpose; K=128 matmul with zeros elsewhere
    wlog1 = np.zeros((128, 96))
    wlog1[84:104, :] = wlog

    expbias = np.zeros((96, 1))
    for r in range(96):
        mk = _rowmap(r)
        if mk is None:
            continue
        d, k = mk
        expbias[r, 0] = math.log(16.0 * COMB15[k]) - colshift[d, k]

    f1w = np.zeros((80, 32))
    for i in (1, 2, 3):
        f1w[(i - 1) * 16 : i * 16, i] = 1.0
    f1w[64:80, 4] = 1.0

    f2w = np.zeros((96, 32))
    f2w[80:96, 0] = Wd[0][0]  # dcoef weights of dim 0 applied to db d=0 rows
    f2w[48, 5:32] = 1.0  # pad cols read dbT row 48 (==1.0): psum pad stays ln-safe

    lnones = np.zeros((128, 4))
    for t in range(4):
        lnones[32 * t : 32 * t + 5, t] = 1.0

    fbias = np.full((4, 1), -DIM * math.log(SC))

    return {
        "fbias": fbias.astype(np.float32),
        "w13": w13.astype(np.float16),
        "w4a": w4a.astype(np.float16),
        "w4b": w4b.astype(np.float16),
        "wlog1": wlog1.astype(np.float16),
        "expbias": expbias.astype(np.float32),
        "f1w": f1w.astype(np.float16),
        "f2w": f2w.astype(np.float16),
        "lnones": lnones.astype(np.float32),
    }


# ---------------------------------------------------------------- device build
def _ap(t, extra_offset, dims):
    """Manual AP over a tile: keep its partition dim, custom free dims."""
    return bass.AP(
        tensor=t.tensor, offset=t.offset + extra_offset, ap=[list(t.ap[0])] + dims
    )


def build_nc(ncore, nblk):
    """nblk = sub-tiles (128 samples each) per block; must be mult of 16."""
    assert nblk % 16 == 0
    nsamp_blk = 128 * nblk
    assert ncore % nsamp_blk == 0
    nblocks = ncore // nsamp_blk
    ngroups = nblk // 4  # 512-sample groups per block
    xcols = ncore // 128 * DIM

    nc = bacc.Bacc("TRN2", target_bir_lowering=False, debug=False, num_devices=N_CORES)
    xt = nc.declare_dram_parameter("xt", [128, xcols], F32, isOutput=False)
    w13 = nc.declare_dram_parameter("w13", [84, 64], F16, isOutput=False)
    w4a = nc.declare_dram_parameter("w4a", [128, 16], F16, isOutput=False)
    w4b = nc.declare_dram_parameter("w4b", [128, 16], F16, isOutput=False)
    wlog1 = nc.declare_dram_parameter("wlog1", [128, 96], F16, isOutput=False)
    expbias = nc.declare_dram_parameter("expbias", [96, 1], F32, isOutput=False)
    f1w = nc.declare_dram_parameter("f1w", [80, 32], F16, isOutput=False)
    f2w = nc.declare_dram_parameter("f2w", [96, 32], F16, isOutput=False)
    lnones = nc.declare_dram_parameter("lnones", [128, 4], F32, isOutput=False)
    fbias = nc.declare_dram_parameter("fbias", [4, 1], F32, isOutput=False)
    dens = nc.declare_dram_parameter("dens", [ncore], F32, isOutput=True)

    Exp = mybir.ActivationFunctionType.Exp
    Ln = mybir.ActivationFunctionType.Ln

    with tile.TileContext(nc) as tc:
        with (
            tc.tile_pool(name="wc", bufs=1) as wc,
            tc.tile_pool(name="la", bufs=2) as la,
            tc.tile_pool(name="gr", bufs=3) as gr,
            tc.tile_pool(name="tr", bufs=2) as tr,
            tc.tile_pool(name="sb", bufs=2) as sbp,
            tc.tile_pool(name="psg", bufs=2, space="PSUM") as psg,
            tc.tile_pool(name="psf", bufs=2, space="PSUM") as psf,
            tc.tile_pool(name="psd", bufs=2, space="PSUM") as psd,
        ):
            w13sb = wc.tile([84, 64], F16, tag="w13")
            w4asb = wc.tile([128, 16], F16, tag="w4a")
            w4bsb = wc.tile([128, 16], F16, tag="w4b")
            wlogsb = wc.tile([128, 96], F16, tag="wlog")
            expbsb = wc.tile([96, 1], F32, tag="expb")
            f1wsb = wc.tile([80, 32], F16, tag="f1w")
            f2wsb = wc.tile([96, 32], F16, tag="f2w")
            lnosb = wc.tile([128, 4], F32, tag="lno")
            fbsb = wc.tile([4, 1], F32, tag="fb")
            xall = wc.tile([128, xcols], F32, tag="xall")
            for dst, src in (
                (w13sb, w13),
                (w4asb, w4a),
                (w4bsb, w4b),
                (wlogsb, wlog1),
                (expbsb, expbias),
                (f1wsb, f1w),
                (f2wsb, f2w),
                (lnosb, lnones),
                (fbsb, fbias),
                (xall, xt),
            ):
                nc.gpsimd.dma_start(out=dst[:], in_=src[:])

            for blk in range(nblocks):
                n = nblk
                xa = xall[:, blk * n * 5 : (blk + 1) * n * 5].rearrange(
                    "p (n d) -> p n d", d=5
                )
                u = la.tile([128, n, 4], F32, tag="u")
                xp2 = la.tile([128, n, 4], F32, tag="xp2")
                up2 = la.tile([128, n, 4], F32, tag="up2")
                ln32 = la.tile([128, n, 10], F32, tag="ln32")
                b4 = la.tile([128, n, 4, 4], F16, tag="b4")
                # per-sub-tile 128-col regions (contiguous -> one block transpose each)
                # cbA: 0:4 cb1 | 4:20 cb2 | 20:84 cb3 | 84:104 ln hi/lo | 104:128 zero
                cbA = la.tile([128, n, 128], F16, tag="cbA")
                cbB = la.tile([128, n, 128], F16, tag="cbB")  # cb4 rows 0:128
                cbC = la.tile([128, n, 128], F16, tag="cbC")  # cb4 rows 128:256

                x4 = xa[:, :, 0:4]
                nc.vector.tensor_scalar(
                    out=u[:],
                    in0=x4,
                    scalar1=1.0,
                    scalar2=-1.0,
                    op0=mybir.AluOpType.subtract,
                    op1=mybir.AluOpType.mult,
                )
                nc.vector.tensor_mul(out=xp2[:], in0=x4, in1=x4)
                nc.vector.tensor_mul(out=up2[:], in0=u[:], in1=u[:])
                nc.scalar.activation(out=ln32[:, :, 0:5], in_=xa, func=Ln)
                nc.scalar.activation(
                    out=ln32[:, :, 5:10], in_=xa, func=Ln, scale=-1.0, bias=1.0
                )
                nc.vector.tensor_copy(out=cbA[:, :, 84:94], in_=ln32[:])
                nc.vector.tensor_sub(
                    out=cbA[:, :, 94:104], in0=ln32[:], in1=cbA[:, :, 84:94]
                )
                nc.vector.memset(cbA[:, :, 104:128], 0.0)
                # b4[:, :, j, d]: j0=u^3, j1=x u^2, j2=x^2 u, j3=x^3 (d=0..3)
                nc.vector.tensor_mul(out=b4[:, :, 0, :], in0=up2[:], in1=u[:])
                nc.vector.tensor_mul(out=b4[:, :, 1, :], in0=x4, in1=up2[:])
                nc.vector.tensor_mul(out=b4[:, :, 2, :], in0=xp2[:], in1=u[:])
                nc.vector.tensor_mul(out=b4[:, :, 3, :], in0=xp2[:], in1=x4)
                nc.vector.tensor_copy(
                    out=cbA[:, :, 0:4], in_=_ap(b4[:], 0, [[16, n], [4, 4]])
                )
                nc.vector.tensor_mul(
                    out=cbA[:, :, 4:20].rearrange("p n (a b) -> p n a b", a=4),
                    in0=_ap(b4[:], 0, [[16, n], [0, 4], [4, 4]]),
                    in1=_ap(b4[:], 1, [[16, n], [4, 4], [0, 4]]),
                )
                nc.vector.tensor_mul(
                    out=cbA[:, :, 20:84].rearrange("p n (a b) -> p n a b", a=4),
                    in0=_ap(cbA[:], 4, [[128, n], [0, 4], [1, 16]]),
                    in1=_ap(b4[:], 2, [[16, n], [4, 4], [0, 16]]),
                )
                nc.vector.tensor_mul(
                    out=cbB[:].rearrange("p n (a b) -> p n a b", a=2),
                    in0=_ap(cbA[:], 20, [[128, n], [0, 2], [1, 64]]),
                    in1=_ap(b4[:], 3, [[16, n], [4, 2], [0, 64]]),
                )
                nc.gpsimd.tensor_mul(
                    out=cbC[:].rearrange("p n (a b) -> p n a b", a=2),
                    in0=_ap(cbA[:], 20, [[128, n], [0, 2], [1, 64]]),
                    in1=_ap(b4[:], 11, [[16, n], [4, 2], [0, 64]]),
                )

                # one batched xbar transpose per region per block:
                # out[:, j, :] = transpose(in[:, j*128:(j+1)*128])
                cbTA = tr.tile([128, n, 128], F16, tag="cbTA")
                cbTB = tr.tile([128, n, 128], F16, tag="cbTB")
                cbTC = tr.tile([128, n, 128], F16, tag="cbTC")
                for dst_t, src_t in ((cbTA, cbA), (cbTB, cbB), (cbTC, cbC)):
                    nc.sync.dma_start(
                        out=dst_t[:],
                        in_=src_t[:].rearrange("p n c -> p (n c)"),
                        transpose=True,
                    )

                for sb in range(ngroups // 4):  # superblock: 4 groups / 2048 samples
                    fpsum = psf.tile([128, 512], F32, tag="fpsum")
                    for tp in range(4):
                        g = sb * 4 + tp
                        gsl = slice(4 * g, 4 * g + 4)
                        wlogp = psg.tile([96, 512], F32, tag="wlogp")
                        for t in range(4):
                            nc.tensor.matmul(
                                out=wlogp[:, t * 128 : (t + 1) * 128],
                                lhsT=wlogsb[:],
                                rhs=cbTA[:, 4 * g + t, :],
                                start=True,
                                stop=True,
                            )
                        dbT = gr.tile([96, 512], F16, tag="dbT")
                        nc.scalar.activation(
                            out=dbT[:], in_=wlogp[:], func=Exp, bias=expbsb[:]
                        )
                        dtfp = psg.tile([96, 512], F32, tag="dtfp")
                        nc.tensor.matmul(
                            out=dtfp[0:64, :],
                            lhsT=w13sb[:],
                            rhs=cbTA[0:84, gsl, :],
                            start=True,
                            stop=True,
                        )
                        nc.tensor.matmul(
                            out=dtfp[64:80, :],
                            lhsT=w4asb[:],
                            rhs=cbTB[:, gsl, :],
                            start=True,
                            stop=False,
                        )
                        nc.tensor.matmul(
                            out=dtfp[64:80, :],
                            lhsT=w4bsb[:],
                            rhs=cbTC[:, gsl, :],
                            start=False,
                            stop=True,
                        )
                        prod = gr.tile([80, 512], F16, tag="prod")
                        nc.vector.tensor_mul(
                            out=prod[:], in0=dtfp[0:80, :], in1=dbT[0:80, :]
                        )
                        frows = slice(32 * tp, 32 * tp + 32)
                        nc.tensor.matmul(
                            out=fpsum[frows, :],
                            lhsT=f1wsb[:],
                            rhs=prod[:],
                            start=True,
                            stop=False,
                            tile_position=(0, 32 * tp),
                        )
                        nc.tensor.matmul(
                            out=fpsum[frows, :],
                            lhsT=f2wsb[:],
                            rhs=dbT[:],
                            start=False,
                            stop=True,
                            tile_position=(0, 32 * tp),
                        )
                    lnf = sbp.tile([128, 512], F32, tag="lnf")
                    nc.scalar.activation(out=lnf[:], in_=fpsum[:], func=Ln)
                    lnden = psd.tile([4, 512], F32, tag="lnden")
                    nc.tensor.matmul(
                        out=lnden[:],
                        lhsT=lnosb[:],
                        rhs=lnf[:],
                        start=True,
                        stop=True,
                    )
                    dens_sb = sbp.tile([4, 512], F32, tag="dens_sb")
                    nc.scalar.activation(
                        out=dens_sb[:],
                        in_=lnden[:],
                        func=Exp,
                        bias=fbsb[:],
                    )
                    base = blk * nsamp_blk + sb * 2048
                    nc.gpsimd.dma_start(
                        out=dens[base : base + 2048].rearrange("(t s) -> t s", t=4),
                        in_=dens_sb[:],
                    )
    nc.finalize()
    return nc


# -------------------------------------------------------------------- host run
def pack_x(x_shard):
    """[N_CORE, 5] -> [128, N_CORE/128*5]; sample s = nb*128+p -> row p, cols nb*5+d."""
    n = x_shard.shape[0]
    return (
        np.ascontiguousarray(x_shard.reshape(n // 128, 128, 5).transpose(1, 0, 2))
        .reshape(128, n // 128 * 5)
        .astype(np.float32)
    )


_CACHE = {}


def _get_runner():
    """Build nc + a cached jitted shard_map callable (trace/compile once)."""
    if "runner" in _CACHE:
        return _CACHE["runner"]
    import jax
    from jax.sharding import Mesh, PartitionSpec
    from jax.experimental.shard_map import shard_map

    from concourse import bass2jax, mybir as _mb
    from concourse.bass2jax import (
        _bass_exec_p,
        install_neuronx_cc_hook,
        partition_id_tensor,
    )

    install_neuronx_cc_hook()
    nc = build_nc(N_CORE, 32)
    partition_name = nc.partition_id_tensor.name if nc.partition_id_tensor else None

    in_names, out_names, out_avals, zero_outs = [], [], [], []
    for alloc in nc.m.functions[0].allocations:
        if not isinstance(alloc, _mb.MemoryLocationSet):
            continue
        name = alloc.memorylocations[0].name
        if alloc.kind == "ExternalInput":
            if name != partition_name:
                in_names.append(name)
        elif alloc.kind == "ExternalOutput":
            out_names.append(name)
            shape = tuple(alloc.tensor_shape)
            dtype = _mb.dt.np(alloc.dtype)
            out_avals.append(jax.core.ShapedArray(shape, dtype))
            zero_outs.append(np.zeros(shape, dtype))
    n_params = len(in_names)
    all_in_names = list(in_names) + list(out_names)
    if partition_name is not None:
        all_in_names.append(partition_name)

    def _body(*args):
        operands = list(args)
        if partition_name is not None:
            operands.append(partition_id_tensor())
        outs = _bass_exec_p.bind(
            *operands,
            out_avals=tuple(out_avals),
            in_names=tuple(all_in_names),
            out_names=tuple(out_names),
            lowering_input_output_aliases=(),
            sim_require_finite=True,
            sim_require_nnan=True,
            nc=nc,
        )
        return tuple(outs)

    devices = jax.devices()[:N_CORES]
    mesh = Mesh(np.asarray(devices), ("core",))
    in_specs = (PartitionSpec("core"),) * (n_params + len(out_names))
    out_specs = (PartitionSpec("core"),) * len(out_names)
    sharded = jax.jit(
        shard_map(
            _body, mesh=mesh, in_specs=in_specs, out_specs=out_specs, check_rep=False
        ),
        keep_unused=True,
    )
    shard = jax.NamedSharding(mesh, PartitionSpec("core"))
    zeros_dev = [
        jax.device_put(
            np.zeros((N_CORES * z.shape[0], *z.shape[1:]), z.dtype), shard
        )
        for z in zero_outs
    ]
    _CACHE["runner"] = (sharded, in_names, out_names, out_avals, zeros_dev, shard)
    return _CACHE["runner"]


def run_device(in_maps):
    """in_maps: per-core dicts. Returns list of per-core output dicts."""
    import jax

    sharded, in_names, out_names, out_avals, zeros_dev, shard = _get_runner()
    concat_in = [
        jax.device_put(
            np.concatenate(
                [np.asarray(in_maps[c][k]) for c in range(N_CORES)], axis=0
            ),
            shard,
        )
        for k in in_names
    ]
    out_arrs = sharded(*concat_in, *zeros_dev)
    return [
        {
            k: np.asarray(out_arrs[i]).reshape(N_CORES, *out_avals[i].shape)[c]
            for i, k in enumerate(out_names)
        }
        for c in range(N_CORES)
    ]


def make_in_maps(x, A_list):
    consts = build_consts([np.asarray(a) for a in A_list])
    in_maps = []
    for c in range(N_CORES):
        m = {"xt": pack_x(x[c * N_CORE : (c + 1) * N_CORE])}
        m.update(consts)
        in_maps.append(m)
    return in_maps


def kernel(x, A0, A1, A2, A3, A4):
    x = np.asarray(x, dtype=np.float32)
    in_maps = make_in_maps(x, (A0, A1, A2, A3, A4))
    res = run_device(in_maps)
    return np.concatenate([res[c]["dens"] for c in range(N_CORES)])



# revision 19
# speedup vs baseline: 132.4565x; 4.7495x over previous
"""Bernstein flow density kernel for Trainium2 (8 NeuronCores, data-parallel).

Math (per sample x in R^5, per dim i):
  c = constrained(A_i)                     # [(4)^i, 15] monotone coeffs in (0,1)
  tf_k = sum_j cb_ij c[j,k]                # cb_i = multivariate Bernstein basis over x[:i]
  dcoef_k = tf_k - tf_{k-1}  (tf_{-1}=0, tf_15=1)
  db_k = 16*comb(15,k) x_i^k (1-x_i)^(15-k)
  f_i = sum_k dcoef_k db_k ;  density = prod_i f_i

Device mapping (one fused pass, basis-major after a single 2-byte DMA
transpose per 4096-sample block):
  - cb over dims 0..2 (rows: 4 cb1 + 16 cb2 + 64 cb3, pure monomials with
    comb(3,.) folded into weights) built on DVE in fp16, plus fp16 hi/lo of
    ln x_d / ln(1-x_d) at rows 84:104 -> one 128-col region -> one transpose.
  - dim 4's 256-row contraction is FACTORED: cb4 = cb3 (x) b(x3), so
    dtf4[(j3,k)] = sum_{p012} cb3 * W4[(j3,p012),k] rides in the same K=84
    matmul that computes dims 1-3 (rows 64:128 of a single [84,128] lhsT),
    and the b(x3) factor folds into the exponential db path.
  - dim 0's dcoef is constant; it enters the same matmul through the
    partition-of-unity of cb1 (rows 48:64).
  - db for all dims + the (j3,k) extension: one K=128 matmul over the ln
    hi/lo rows -> [128,512] exponents -> one ACT Exp (bias folds the
    2^e fp16-range compensation and ln(16*comb)).
  - f_i = one 0/1-weight matmul over prod = dtf*db; density = exp(ones-matmul
    of ln f).  Exp and Ln share one activation table set (pinned below) so
    the ACT engine never reloads tables in steady state.
"""

import math
import sys

import numpy as np

for _p in ("/opt/trn_rl_repo", "/root/.axon_site/_ro/trn_rl_repo"):
    if _p not in sys.path:
        sys.path.append(_p)

import concourse.bass as bass
import concourse.tile as tile
from concourse import bacc, mybir
from concourse import hw_specs as _hw_specs

F32 = mybir.dt.float32
F16 = mybir.dt.float16

DIM = 5
TF_DEG = 16
N_FULL = 262144
N_CORES = 8
N_CORE = N_FULL // N_CORES  # 32768
SC = 256.0  # scale folded into dcoef weights to keep fp16 away from subnormals
LN2 = math.log(2.0)
COMB3 = np.array([1.0, 3.0, 3.0, 1.0])
COMB15 = np.array([math.comb(15, k) for k in range(16)], dtype=np.float64)

# ---------------------------------------------------------------- act tables
# Exp and Ln coexist in the 'natural_log_exp_and_others' table set. The
# default chooser pairs each function with the first set containing it, which
# makes interleaved Exp/Ln reload the table (1283 ns each) every switch.
# Narrow the chooser's options so exp/ln resolve to the combined set (set ids
# keep their act_info.json positions, so the emitted NEFF stays valid).
_PINNED = "natural_log_exp_and_others"
_EXPLN = {mybir.ActivationFunctionType.Exp, mybir.ActivationFunctionType.Ln}


def _pinned_get_activation_tables(arch):
    base = _hw_specs.get_activation_tables(arch)
    out = {}
    for name, s in base.items():
        if name == _PINNED and _EXPLN <= s:
            out[name] = set(s)
        else:
            out[name] = {f for f in s if f not in _EXPLN}
    if not any(_EXPLN <= s for s in out.values()):
        return {k: set(v) for k, v in base.items()}  # fall back untouched
    return out


bacc.get_activation_tables = _pinned_get_activation_tables


# ----------------------------------------------------------------- host consts
def _constrained(A):
    A = A.astype(np.float64)
    sp = np.log1p(np.exp(-np.abs(A))) + np.maximum(A, 0.0)  # softplus, stable
    cs = np.cumsum(sp, axis=1)
    return 2.0 * (1.0 / (1.0 + np.exp(-cs)) - 0.5)


def _dev_perm_scale(i):
    """Map device row p (p = sum_d j_d*4^d, j_0 fastest) to reference row
    (ref = sum_d j_d*4^(i-1-d), j_0 slowest) + comb(3,.) product scale."""
    rows = 4**i
    ref_idx = np.zeros(rows, dtype=np.int64)
    scale = np.ones(rows)
    for p in range(rows):
        r = 0
        s = 1.0
        for d in range(i):
            jd = (p >> (2 * d)) & 3
            r += jd * 4 ** (i - 1 - d)
            s *= COMB3[jd]
        ref_idx[p] = r
        scale[p] = s
    return ref_idx, scale


def _dcoef_weights(C, combscale):
    """C: [rows,15] device-row-ordered coeffs; returns [rows,16] W with the
    tf-difference folded in, scaled so sum_j monomial_j W[j,k] = SC*dcoef_k."""
    rows = C.shape[0]
    W = np.zeros((rows, 16))
    W[:, 0] = C[:, 0]
    W[:, 1:15] = C[:, 1:15] - C[:, 0:14]
    W[:, 15] = 1.0 - C[:, 14]
    return W * combscale[:, None] * SC


def _range_scale(W):
    """Per-column power-of-two scale placing max|W| near 1024 (fp16 range);
    returns scaled W and ln of the applied scale (for exp-bias compensation)."""
    m = np.max(np.abs(W), axis=0)
    e = np.clip(np.round(np.log2(1024.0 / np.maximum(m, 1e-300))), -10, 40)
    return W * np.exp2(e)[None, :], e * LN2


def build_consts(A_list):
    Cs = []
    for i in range(DIM):
        C = _constrained(A_list[i])
        if i == 0:
            Cs.append((C, np.ones(1)))
        else:
            ref_idx, scale = _dev_perm_scale(i)
            Cs.append((C[ref_idx], scale))
    Wd = [_dcoef_weights(Cp, sc) for (Cp, sc) in Cs]  # [4^i,16] each, SC folded

    colshift = np.zeros((4, 16))
    for i in range(4):  # dims 0..3: per-(d,k) fp16 range scaling
        Wd[i], colshift[i] = _range_scale(Wd[i])

    # dim 4 factored: device row p = j3*64 + p012 -> W4f[p012, j3*16+k]
    W4 = Wd[4].reshape(4, 64, 16)  # [j3, p012, k]
    W4f = np.transpose(W4, (1, 0, 2)).reshape(64, 64)
    W4f, colshift4 = _range_scale(W4f)  # [64] over (j3,k)

    # Single dtf lhsT [84,128]: psum rows 0:16 d1 | 16:32 d2 | 32:48 d3 |
    # 48:64 d0 (constant dcoef via cb1 partition of unity) | 64:128 d4 (j3,k)
    w13 = np.zeros((84, 128))
    w13[0:4, 0:16] = Wd[1]
    w13[4:20, 16:32] = Wd[2]
    w13[20:84, 32:48] = Wd[3]
    w13[0:4, 48:64] = COMB3[:, None] * Wd[0][0][None, :]
    w13[20:84, 64:128] = W4f

    # db rows r aligned with dtf rows: exponent-sum weights over the ln
    # hi/lo rows (84:94 hi, 94:104 lo after the block transpose).
    wlog1 = np.zeros((128, 128))
    expbias = np.zeros((128, 1))

    def _terms(r):
        # returns ([(lnx_d, coeff), (lnu_d, coeff), ...], bias)
        if r < 48:
            d, k = 1 + r // 16, r % 16
            return [(d, float(k)), (5 + d, float(15 - k))], (
                math.log(16.0 * COMB15[k]) - colshift[d][k]
            )
        if r < 64:
            k = r - 48
            return [(0, float(k)), (5, float(15 - k))], (
                math.log(16.0 * COMB15[k]) - colshift[0][k]
            )
        j3, k = divmod(r - 64, 16)
        return [
            (3, float(j3)),
            (8, float(3 - j3)),
            (4, float(k)),
            (9, float(15 - k)),
        ], (math.log(16.0 * COMB15[k]) - colshift4[j3 * 16 + k])

    for r in range(128):
        terms, bias = _terms(r)
        for idx, coeff in terms:
            wlog1[84 + idx, r] += coeff  # hi rows
            wlog1[94 + idx, r] += coeff  # lo rows
        expbias[r, 0] = bias

    # f contraction: prod rows -> f_i columns; cols 5:32 replicate f_0 so the
    # PSUM pad rows stay strictly positive for the Ln (ignored by lnones).
    f1w = np.zeros((128, 32))
    f1w[0:16, 1] = 1.0
    f1w[16:32, 2] = 1.0
    f1w[32:48, 3] = 1.0
    f1w[48:64, 0] = 1.0
    f1w[64:128, 4] = 1.0
    f1w[48:64, 5:32] = 1.0

    lnones = np.zeros((128, 4))
    for t in range(4):
        lnones[32 * t : 32 * t + 5, t] = 1.0

    fbias = np.full((4, 1), -DIM * math.log(SC))

    return {
        "fbias": fbias.astype(np.float32),
        "w13": w13.astype(np.float16),
        "wlog1": wlog1.astype(np.float16),
        "expbias": expbias.astype(np.float32),
        "f1w": f1w.astype(np.float16),
        "lnones": lnones.astype(np.float32),
    }


# ---------------------------------------------------------------- device build
def _ap(t, extra_offset, dims):
    """Manual AP over a tile: keep its partition dim, custom free dims."""
    return bass.AP(
        tensor=t.tensor, offset=t.offset + extra_offset, ap=[list(t.ap[0])] + dims
    )


def build_nc(ncore, nblk, nrep=1, la_bufs=2, tr_bufs=2, gr_bufs=3, sb_bufs=2):
    """nblk = sub-tiles (128 samples each) per block; must be mult of 16.
    nrep > 1 wraps the whole computation in a hardware loop that recomputes
    the identical output nrep times (used only to measure steady-state
    device execution time without per-launch overhead)."""
    assert nblk % 16 == 0
    nsamp_blk = 128 * nblk
    assert ncore % nsamp_blk == 0
    nblocks = ncore // nsamp_blk
    ngroups = nblk // 4  # 512-sample groups per block
    xcols = ncore // 128 * DIM

    nc = bacc.Bacc("TRN2", target_bir_lowering=False, debug=False, num_devices=N_CORES)
    xt = nc.declare_dram_parameter("xt", [128, xcols], F32, isOutput=False)
    w13 = nc.declare_dram_parameter("w13", [84, 128], F16, isOutput=False)
    wlog1 = nc.declare_dram_parameter("wlog1", [128, 128], F16, isOutput=False)
    expbias = nc.declare_dram_parameter("expbias", [128, 1], F32, isOutput=False)
    f1w = nc.declare_dram_parameter("f1w", [128, 32], F16, isOutput=False)
    lnones = nc.declare_dram_parameter("lnones", [128, 4], F32, isOutput=False)
    fbias = nc.declare_dram_parameter("fbias", [4, 1], F32, isOutput=False)
    dens = nc.declare_dram_parameter("dens", [ncore], F32, isOutput=True)

    Exp = mybir.ActivationFunctionType.Exp
    Ln = mybir.ActivationFunctionType.Ln

    with tile.TileContext(nc) as tc:
        with (
            tc.tile_pool(name="wc", bufs=1) as wc,
            tc.tile_pool(name="la", bufs=la_bufs) as la,
            tc.tile_pool(name="gr", bufs=gr_bufs) as gr,
            tc.tile_pool(name="tr", bufs=tr_bufs) as tr,
            tc.tile_pool(name="sb", bufs=sb_bufs) as sbp,
            tc.tile_pool(name="psw", bufs=2, space="PSUM") as psw,
            tc.tile_pool(name="psg", bufs=2, space="PSUM") as psg,
            tc.tile_pool(name="psf", bufs=2, space="PSUM") as psf,
            tc.tile_pool(name="psd", bufs=2, space="PSUM") as psd,
        ):
            w13sb = wc.tile([84, 128], F16, tag="w13")
            wlogsb = wc.tile([128, 128], F16, tag="wlog")
            expbsb = wc.tile([128, 1], F32, tag="expb")
            f1wsb = wc.tile([128, 32], F16, tag="f1w")
            lnosb = wc.tile([128, 4], F32, tag="lno")
            fbsb = wc.tile([4, 1], F32, tag="fb")
            xall = wc.tile([128, xcols], F32, tag="xall")
            for dst, src in (
                (w13sb, w13),
                (wlogsb, wlog1),
                (expbsb, expbias),
                (f1wsb, f1w),
                (lnosb, lnones),
                (fbsb, fbias),
                (xall, xt),
            ):
                nc.gpsimd.dma_start(out=dst[:], in_=src[:])

            import contextlib

            rep_ctx = tc.For_i(0, nrep) if nrep > 1 else contextlib.nullcontext()
            with rep_ctx:
                for blk in range(nblocks):
                    n = nblk
                    xa = xall[:, blk * n * 5 : (blk + 1) * n * 5].rearrange(
                        "p (n d) -> p n d", d=5
                    )
                    u3 = la.tile([128, n, 3], F32, tag="u3")
                    xp2 = la.tile([128, n, 3], F32, tag="xp2")
                    up2 = la.tile([128, n, 3], F32, tag="up2")
                    ln32 = la.tile([128, n, 10], F32, tag="ln32")
                    # b4 free dims (n, j, d): monomials mono_d(j) for d=0..2
                    b4 = la.tile([128, n, 4, 3], F16, tag="b4")
                    # cbA cols: 0:4 cb1 | 4:20 cb2 | 20:84 cb3 | 84:104 ln
                    # hi/lo | 104:128 zero
                    cbA = la.tile([128, n, 128], F16, tag="cbA")

                    x3 = xa[:, :, 0:3]
                    nc.vector.tensor_scalar(
                        out=u3[:],
                        in0=x3,
                        scalar1=1.0,
                        scalar2=-1.0,
                        op0=mybir.AluOpType.subtract,
                        op1=mybir.AluOpType.mult,
                    )
                    nc.vector.tensor_mul(out=xp2[:], in0=x3, in1=x3)
                    nc.vector.tensor_mul(out=up2[:], in0=u3[:], in1=u3[:])
                    nc.scalar.activation(out=ln32[:, :, 0:5], in_=xa, func=Ln)
                    nc.scalar.activation(
                        out=ln32[:, :, 5:10], in_=xa, func=Ln, scale=-1.0, bias=1.0
                    )
                    nc.vector.tensor_copy(out=cbA[:, :, 84:94], in_=ln32[:])
                    nc.gpsimd.tensor_sub(
                        out=cbA[:, :, 94:104], in0=ln32[:], in1=cbA[:, :, 84:94]
                    )
                    nc.gpsimd.memset(cbA[:, :, 104:128], 0.0)
                    # b4[:, :, j, d]: j0=u^3, j1=x u^2, j2=x^2 u, j3=x^3
                    nc.gpsimd.tensor_mul(out=b4[:, :, 0, :], in0=up2[:], in1=u3[:])
                    nc.gpsimd.tensor_mul(out=b4[:, :, 1, :], in0=x3, in1=up2[:])
                    nc.gpsimd.tensor_mul(out=b4[:, :, 2, :], in0=xp2[:], in1=u3[:])
                    nc.gpsimd.tensor_mul(out=b4[:, :, 3, :], in0=xp2[:], in1=x3)
                    nc.vector.tensor_copy(
                        out=cbA[:, :, 0:4], in_=_ap(b4[:], 0, [[12, n], [3, 4]])
                    )
                    nc.gpsimd.tensor_mul(
                        out=cbA[:, :, 4:20].rearrange("p n (a b) -> p n a b", a=4),
                        in0=_ap(b4[:], 0, [[12, n], [0, 4], [3, 4]]),
                        in1=_ap(b4[:], 1, [[12, n], [3, 4], [0, 4]]),
                    )
                    nc.gpsimd.tensor_mul(
                        out=cbA[:, :, 20:84].rearrange("p n (a b) -> p n a b", a=4),
                        in0=_ap(cbA[:], 4, [[128, n], [0, 4], [1, 16]]),
                        in1=_ap(b4[:], 2, [[12, n], [3, 4], [0, 16]]),
                    )

                    # one batched xbar transpose per block:
                    # cbTA[p, j, q] = cbA[q, j, p]
                    cbTA = tr.tile([128, n, 128], F16, tag="cbTA")
                    nc.sync.dma_start(
                        out=cbTA[:],
                        in_=cbA[:].rearrange("p n c -> p (n c)"),
                        transpose=True,
                    )

                    for g in range(ngroups):
                        tp = g % 4
                        gsl = slice(4 * g, 4 * g + 4)
                        wlogp = psw.tile([128, 512], F32, tag="wlogp")
                        nc.tensor.matmul(
                            out=wlogp[:],
                            lhsT=wlogsb[:],
                            rhs=cbTA[:, gsl, :],
                            start=True,
                            stop=True,
                        )
                        dbT = gr.tile([128, 512], F16, tag="dbT")
                        nc.scalar.activation(
                            out=dbT[:], in_=wlogp[:], func=Exp, bias=expbsb[:]
                        )
                        dtfp = psg.tile([128, 512], F32, tag="dtfp")
                        nc.tensor.matmul(
                            out=dtfp[:],
                            lhsT=w13sb[:],
                            rhs=cbTA[0:84, gsl, :],
                            start=True,
                            stop=True,
                        )
                        prod = gr.tile([128, 512], F16, tag="prod")
                        nc.vector.tensor_mul(out=prod[:], in0=dtfp[:], in1=dbT[:])
                        if tp == 0:
                            fpsum = psf.tile([128, 512], F32, tag="fpsum")
                        nc.tensor.matmul(
                            out=fpsum[32 * tp : 32 * tp + 32, :],
                            lhsT=f1wsb[:],
                            rhs=prod[:],
                            start=True,
                            stop=True,
                            tile_position=(0, 32 * tp),
                        )
                        if tp == 3:
                            lnf = sbp.tile([128, 512], F32, tag="lnf")
                            nc.scalar.activation(out=lnf[:], in_=fpsum[:], func=Ln)
                            lnden = psd.tile([4, 512], F32, tag="lnden")
                            nc.tensor.matmul(
                                out=lnden[:],
                                lhsT=lnosb[:],
                                rhs=lnf[:],
                                start=True,
                                stop=True,
                            )
                            dens_sb = sbp.tile([4, 512], F32, tag="dens_sb")
                            nc.scalar.activation(
                                out=dens_sb[:],
                                in_=lnden[:],
                                func=Exp,
                                bias=fbsb[:],
                            )
                            base = blk * nsamp_blk + (g // 4) * 2048
                            nc.sync.dma_start(
                                out=dens[base : base + 2048].rearrange(
                                    "(t s) -> t s", t=4
                                ),
                                in_=dens_sb[:],
                            )
    nc.finalize()
    return nc


# -------------------------------------------------------------------- host run
def pack_x(x_shard):
    """[N_CORE, 5] -> [128, N_CORE/128*5]; sample s = nb*128+p -> row p, cols nb*5+d."""
    n = x_shard.shape[0]
    return (
        np.ascontiguousarray(x_shard.reshape(n // 128, 128, 5).transpose(1, 0, 2))
        .reshape(128, n // 128 * 5)
        .astype(np.float32)
    )


_CACHE = {}


def _get_runner(nrep=1):
    """Build nc + a cached jitted shard_map callable (trace/compile once)."""
    key = ("runner", nrep)
    if key in _CACHE:
        return _CACHE[key]
    import jax
    from jax.sharding import Mesh, PartitionSpec
    from jax.experimental.shard_map import shard_map

    from concourse import mybir as _mb
    from concourse.bass2jax import (
        _bass_exec_p,
        install_neuronx_cc_hook,
        partition_id_tensor,
    )

    install_neuronx_cc_hook()
    nc = build_nc(N_CORE, 32, nrep=nrep)
    partition_name = nc.partition_id_tensor.name if nc.partition_id_tensor else None

    in_names, out_names, out_avals, zero_outs = [], [], [], []
    for alloc in nc.m.functions[0].allocations:
        if not isinstance(alloc, _mb.MemoryLocationSet):
            continue
        name = alloc.memorylocations[0].name
        if alloc.kind == "ExternalInput":
            if name != partition_name:
                in_names.append(name)
        elif alloc.kind == "ExternalOutput":
            out_names.append(name)
            shape = tuple(alloc.tensor_shape)
            dtype = _mb.dt.np(alloc.dtype)
            out_avals.append(jax.core.ShapedArray(shape, dtype))
            zero_outs.append(np.zeros(shape, dtype))
    n_params = len(in_names)
    all_in_names = list(in_names) + list(out_names)
    if partition_name is not None:
        all_in_names.append(partition_name)

    def _body(*args):
        operands = list(args)
        if partition_name is not None:
            operands.append(partition_id_tensor())
        outs = _bass_exec_p.bind(
            *operands,
            out_avals=tuple(out_avals),
            in_names=tuple(all_in_names),
            out_names=tuple(out_names),
            lowering_input_output_aliases=(),
            sim_require_finite=True,
            sim_require_nnan=True,
            nc=nc,
        )
        return tuple(outs)

    devices = jax.devices()[:N_CORES]
    mesh = Mesh(np.asarray(devices), ("core",))
    in_specs = (PartitionSpec("core"),) * (n_params + len(out_names))
    out_specs = (PartitionSpec("core"),) * len(out_names)
    sharded = jax.jit(
        shard_map(
            _body, mesh=mesh, in_specs=in_specs, out_specs=out_specs, check_rep=False
        ),
        keep_unused=True,
    )
    shard = jax.NamedSharding(mesh, PartitionSpec("core"))
    zeros_dev = [
        jax.device_put(
            np.zeros((N_CORES * z.shape[0], *z.shape[1:]), z.dtype), shard
        )
        for z in zero_outs
    ]
    _CACHE[key] = (sharded, in_names, out_names, out_avals, zeros_dev, shard)
    return _CACHE[key]


def run_device(in_maps):
    """in_maps: per-core dicts. Returns list of per-core output dicts."""
    import jax

    sharded, in_names, out_names, out_avals, zeros_dev, shard = _get_runner()
    concat_in = [
        jax.device_put(
            np.concatenate(
                [np.asarray(in_maps[c][k]) for c in range(N_CORES)], axis=0
            ),
            shard,
        )
        for k in in_names
    ]
    out_arrs = sharded(*concat_in, *zeros_dev)
    return [
        {
            k: np.asarray(out_arrs[i]).reshape(N_CORES, *out_avals[i].shape)[c]
            for i, k in enumerate(out_names)
        }
        for c in range(N_CORES)
    ]


def make_in_maps(x, A_list):
    consts = build_consts([np.asarray(a) for a in A_list])
    in_maps = []
    for c in range(N_CORES):
        m = {"xt": pack_x(x[c * N_CORE : (c + 1) * N_CORE])}
        m.update(consts)
        in_maps.append(m)
    return in_maps


def kernel(x, A0, A1, A2, A3, A4):
    x = np.asarray(x, dtype=np.float32)
    in_maps = make_in_maps(x, (A0, A1, A2, A3, A4))
    res = run_device(in_maps)
    return np.concatenate([res[c]["dens"] for c in range(N_CORES)])
